# revision 1
# baseline (speedup 1.0000x reference)
"""BitLinear (ternary 2-bit weights, group-128 scales, dynamic int8 activation
quant) for Trainium2, tensor-parallel over 8 NeuronCores (shard N).

Math (per core, N-shard NS):
  s[m]   = 127 / clip(max_k |x[m,k]|, 1e-5)
  q[m,k] = round(x[m,k] * s[m])                      (integers in [-127,127])
  w[n,k] in {-1,0,1} from 2-bit codes c=w+1, 4 codes/byte
  out[m,n] = (sum_k q[m,k] * w[n,k] * ws[n, k//128]) / s[m]   -> bf16

Device scheme: weights staged host-side as uint16 (8 codes each), TRANSPOSED
to [KH=K/8, NS] so the contraction dim lands on SBUF partitions.  Per bit
position t, ONE DVE tensor_scalar extracts c_t = (w16 >> 2t) & 3 (a legal
bitVec op pair; bitVec cannot cast, so the output stays uint16 and the
2-byte dtypes keep DVE fast modes), and one DVE tensor_tensor multiplies by
the host-expanded per-group scale (exact in bf16: c is 0/1/2).  The GEMM
accumulates q @ (c*s).T over 8 bit-planes x 8 kh-blocks into 4 persistent
PSUM tiles; the missing "-1" becomes a tiny rank-64 correction
out -= gsq @ ws.T computed on the PE from the transposed-q planes via a
host-staged group indicator (gmat), with -ws.T staged fp32 (pre-rounded
through bf16 so the scale rounding cancels exactly).

Activation path: row abs-max (DVE reduce as x streams), s = 127*recip(max),
RNE rounding via fma(x, s, 2^23) - 2^23 (ACT then ACT), transpose of q into
per-bit-plane [kh, m] buffers via plain PE matmuls against an identity rhs
(stride-8 lhsT slices pick each bit-plane's columns).  Dummy matmuls tied to
x-chunk arrivals keep the PE HAM clock at 2.4 GHz before real matmuls begin.
DMA-crossbar transposes and fusing extract+scale in one op were measured or
analyzed to be slower (xbar/copy mode serialization; scalar_tensor_tensor
and custom-DVE ops run 1x).
"""

import sys

import numpy as np

try:
    import concourse.bass as bass
except ImportError:  # fresh grading dir: fall back to the repo checkout
    sys.path.insert(0, "/opt/trn_rl_repo")
    import concourse.bass as bass

import ml_dtypes

import concourse.mybir as mybir
import concourse.tile as tile
from concourse import bacc, bass_utils
from concourse.masks import make_identity

FP32 = mybir.dt.float32
BF16 = mybir.dt.bfloat16
U16 = mybir.dt.uint16
MAGIC = float(2 << 22)  # 2^23

M, N, K, GS = 256, 8192, 8192, 128
NCORES = 8


def build_nc(m=M, k=K, ns=N // NCORES):
    """One core's program: full m,k; n-shard of size ns."""
    kh = k // 8          # uint16 count along K
    kb = kh // 128       # kh-blocks of 128 partitions
    st_n = kb // 2       # supertiles = pairs of kh-blocks
    mt = m // 128        # m partition-tiles
    ck = k // 2048       # 2048-wide k-chunks for quant (256 kh, 2 kh-blocks)
    nsl = min(512, ns)   # matmul rhs free-dim slice
    nh_n = ns // nsl
    g_n = k // GS        # scale groups along K

    nc = bacc.Bacc()
    x_d = nc.declare_dram_parameter("x", [m, k], FP32, isOutput=False)
    w_d = nc.declare_dram_parameter("w16", [kh, ns], U16, isOutput=False)
    se_d = nc.declare_dram_parameter("sexp", [kh, ns], BF16, isOutput=False)
    # -ws.T (fp32, pre-rounded through bf16) for the "-1" correction matmul
    sn_d = nc.declare_dram_parameter("sneg", [g_n, ns], FP32, isOutput=False)
    # G[kh, 8*t+gl] = 4^t * (kh//16 == gl): group-sum indicator, bf16
    gm_d = nc.declare_dram_parameter("gmat", [128, 64], BF16, isOutput=False)
    out_d = nc.declare_dram_parameter("out", [m, ns], BF16, isOutput=True)

    x_r = x_d.rearrange("(T p) k -> T p k", p=128)          # [mt,128,k]
    w_r = w_d.rearrange("(B p) n -> p B n", p=128)          # [128,kb,ns]
    se_r = se_d.rearrange("(B p) n -> p B n", p=128)        # [128,kb,ns]
    out_r = out_d.rearrange("(T p) n -> T p n", p=128)      # [mt,128,ns]

    with tile.TileContext(nc) as tc:
        with (
            tc.tile_pool(name="const", bufs=1) as constp,
            tc.tile_pool(name="stat", bufs=1) as statp,
            tc.tile_pool(name="qp", bufs=1) as qpp,
            tc.tile_pool(name="wse", bufs=3) as wsep,
            tc.tile_pool(name="cw", bufs=4) as cwp,
            tc.tile_pool(name="ob", bufs=4) as obp,
            tc.tile_pool(name="psx", bufs=2, space="PSUM") as psxp,
            tc.tile_pool(name="psm", bufs=1, space="PSUM") as psmp,
        ):
            ident = constp.tile([128, 128], BF16, tag="ident")
            make_identity(nc, ident)
            identf = constp.tile([128, 128], FP32, tag="identf")
            make_identity(nc, identf)
            gmat = constp.tile([128, 64], BF16, tag="gmat")
            nc.sync.dma_start(gmat[:], gm_d[:])
            sneg = constp.tile([g_n, ns], FP32, tag="sneg")
            nc.sync.dma_start(sneg[:], sn_d[:])

            def warm(dep_fp32_128x128, n_mm=1):
                """Dummy matmuls reading an already-landed fp32 tile: keep the
                PE HAM activity monitor from re-throttling to 1.2 GHz."""
                for j in range(n_mm):
                    wp = psxp.tile([128, 128], FP32, tag="psx", name=f"wrm{j}")
                    nc.tensor.matmul(
                        wp[:], dep_fp32_128x128, identf[:, :128],
                        start=True, stop=True,
                    )

            qp = [
                qpp.tile([128, 256 * kb], BF16, tag=f"qp{t}", name=f"qp{t}")
                for t in range(8)
            ]
            pre_w = {}

            def load_st(sti):
                wt = wsep.tile([128, 2 * ns], U16, tag="w16", name="wt")
                se = wsep.tile([128, 2 * ns], BF16, tag="sexp", name="se")
                wt3 = wt.rearrange("p (B n) -> p B n", B=2)
                se3 = se.rearrange("p (B n) -> p B n", B=2)
                nc.sync.dma_start(wt3[:], w_r[:, 2 * sti : 2 * sti + 2, :])
                nc.sync.dma_start(se3[:], se_r[:, 2 * sti : 2 * sti + 2, :])
                pre_w[sti] = (wt, se)
            psm = [
                [
                    psmp.tile([128, nsl], FP32, tag=f"ps{mh}{nh}",
                              name=f"ps{mh}{nh}")
                    for nh in range(nh_n)
                ]
                for mh in range(mt)
            ]

            with (
                tc.tile_pool(name="xp", bufs=1) as xp,
                tc.tile_pool(name="qc", bufs=2) as qcp,
                tc.tile_pool(name="t1", bufs=2) as t1p,
                tc.tile_pool(name="pst", bufs=2, space="PSUM") as pstp,
            ):
                # ---- phase A: load x, row abs-max, scales ----
                xsb = [
                    [
                        xp.tile([128, 2048], FP32, tag=f"x{t}c{c}",
                                name=f"x{t}c{c}")
                        for c in range(ck)
                    ]
                    for t in range(mt)
                ]
                rpart = [statp.tile([128, ck], FP32, tag=f"rp{t}", name=f"rp{t}")
                         for t in range(mt)]
                rmax = [statp.tile([128, 1], FP32, tag=f"rm{t}", name=f"rm{t}")
                        for t in range(mt)]
                s_pp = [statp.tile([128, 1], FP32, tag=f"sp{t}", name=f"sp{t}")
                        for t in range(mt)]
                r1s = [statp.tile([128, 1], FP32, tag=f"rs{t}", name=f"rs{t}")
                       for t in range(mt)]
                for mh in range(mt):
                    for c in range(ck):
                        sl = slice(2048 * c, 2048 * (c + 1))
                        nc.sync.dma_start(xsb[mh][c][:], x_r[mh, :, sl])
                        nc.vector.tensor_reduce(
                            rpart[mh][:, c : c + 1], xsb[mh][c][:],
                            axis=mybir.AxisListType.X, op=mybir.AluOpType.max,
                            apply_absolute_value=True,
                        )
                        # PE keep-warm: small early ramp dummies, then a
                        # dense block of long fp32 matmuls that spans the
                        # rowmax tail so HAM stays at 2.4GHz when the real
                        # matmul stream begins.
                        if mh == 0:
                            warm(xsb[mh][c][:, :128], n_mm=2)
                        if (mh, c) == (min(1, mt - 1), 0):
                            for j in range(9):
                                wp = psxp.tile([128, 512], FP32, tag="psx",
                                               name=f"wrmbig{j}")
                                nc.tensor.matmul(
                                    wp[:], xsb[mh][c][:, :128],
                                    xsb[mh][c][:, :512],
                                    start=True, stop=True,
                                )
                    nc.vector.tensor_reduce(
                        rmax[mh][:], rpart[mh][:],
                        axis=mybir.AxisListType.X, op=mybir.AluOpType.max,
                    )
                    nc.vector.tensor_scalar_max(rmax[mh][:], rmax[mh][:], 1e-5)
                    nc.vector.reciprocal(s_pp[mh][:], rmax[mh][:])
                    nc.vector.tensor_scalar_mul(s_pp[mh][:], s_pp[mh][:], 127.0)
                    nc.vector.tensor_scalar_mul(r1s[mh][:], rmax[mh][:],
                                                1.0 / 127.0)

                # ---- phase A2: quantize + transpose q into bit-plane bufs ----
                # qp[t][kh, 256b+128mh+mm] = q[128mh+mm, 8*(128b+kh)+t]*esc(t)
                for c in range(ck):
                    qcs = []
                    for mh in range(mt):
                        t1 = t1p.tile([128, 2048], FP32, tag="t1")
                        # fma(x, s, 2^23) - 2^23 rounds x*s to the nearest
                        # integer (RNE), matching jnp.round up to a ~2^-24
                        # double-rounding corner; on ACT since DVE is the
                        # steady-state bottleneck.
                        nc.scalar.activation(
                            t1[:], xsb[mh][c][:],
                            mybir.ActivationFunctionType.Copy,
                            bias=MAGIC, scale=s_pp[mh][:],
                        )
                        if c == 0:
                            warm(t1[:, :128], n_mm=4)
                        qc = qcp.tile([128, 2048], BF16, tag=f"q{mh}")
                        nc.scalar.activation(
                            qc[:], t1[:],
                            mybir.ActivationFunctionType.Copy, bias=-MAGIC,
                        )
                        qcs.append(qc)
                    for bh2 in range(2):  # kh-block b = 2c + bh2
                        b = 2 * c + bh2
                        for t in range(8):
                            psT = pstp.tile([128, 128 * mt], FP32, tag="psT")
                            for mh in range(mt):
                                # free idx f = 1024*B + 8*kk + t
                                qv = qcs[mh].rearrange(
                                    "p (B kk t) -> p B t kk", B=2, kk=128, t=8
                                )
                                nc.tensor.matmul(
                                    psT[:, 128 * mh : 128 * (mh + 1)],
                                    qv[:, bh2, t, :], ident[:],
                                    start=True, stop=True,
                                )
                            nc.scalar.activation(
                                qp[t][:, 256 * b : 256 * (b + 1)], psT[:],
                                mybir.ActivationFunctionType.Copy,
                            )

            # ---- phase B: weight decode + main matmuls (per supertile) ----
            def phase_b(sti, last_st):
                if sti in pre_w:
                    wt, se = pre_w[sti]
                else:
                    load_st(sti)
                    wt, se = pre_w[sti]
                for t in range(8):
                    # one bitVec tensor_scalar: c_t = (w16 >> 2t) & 3
                    # (bitwise+bitwise pairs are legal; no cast, so out stays
                    # uint16 and the tensor_tensor multiply does the convert)
                    cp = cwp.tile([128, 2 * ns], U16, tag="cp", name="cp")
                    nc.vector.tensor_scalar(
                        cp[:], wt[:], 2 * t, 3,
                        mybir.AluOpType.logical_shift_right,
                        mybir.AluOpType.bitwise_and,
                    )
                    ws = cwp.tile([128, 2 * ns], BF16, tag="ws", name="ws")
                    nc.vector.tensor_tensor(ws[:], cp[:], se[:],
                                            mybir.AluOpType.mult)
                    for bh in range(2):
                        b = 2 * sti + bh
                        first = sti == 0 and t == 0 and bh == 0
                        last = last_st and t == 7 and bh == 1
                        for mh in range(mt):
                            lhsT = qp[t][:, 256 * b + 128 * mh :][:, :128]
                            for nh in range(nh_n):
                                nc.tensor.matmul(
                                    psm[mh][nh][:],
                                    lhsT,
                                    ws[:, ns * bh + nsl * nh :][:, :nsl],
                                    start=first, stop=last,
                                )

            # ---- phase B2: "-1" correction:  out -= sum_g gsq[m,g]*ws[n,g]
            # gsq[m,g] = sum_{k in g} q[m,k] from the qp planes via gmat
            # (undoes the 4^-t evac scaling); groups land on the free dim
            # (PSUM partition bases must be 32-aligned), then a small fp32 PE
            # transpose puts them on partitions.  Runs before the last
            # supertile so the PE tail after DVE finishes is short.
            gsq = constp.tile([8 * kb, 128 * mt], FP32, tag="gsq")

            def b2_chain():
                psgm = [
                    psxp.tile([128, 8 * kb], FP32, tag="psx", name=f"psgm{mh}")
                    for mh in range(mt)
                ]
                for mh in range(mt):
                    for b in range(kb):
                        for t in range(8):
                            nc.tensor.matmul(
                                psgm[mh][:, 8 * b : 8 * b + 8],
                                qp[t][:, 256 * b + 128 * mh :][:, :128],
                                gmat[:, 8 * t : 8 * t + 8],
                                start=(t == 0), stop=(t == 7),
                            )
                gsqm = [
                    constp.tile([128, 8 * kb], FP32, tag=f"gsqm{mh}",
                                name=f"gsqm{mh}")
                    for mh in range(mt)
                ]
                psg = psxp.tile([8 * kb, 128 * mt], FP32, tag="psx", name="psg")
                for mh in range(mt):
                    nc.scalar.activation(
                        gsqm[mh][:], psgm[mh][:],
                        mybir.ActivationFunctionType.Copy,
                    )
                    nc.tensor.matmul(
                        psg[:, 128 * mh : 128 * (mh + 1)],
                        gsqm[mh][:], identf[:, :128],
                        start=True, stop=True,
                    )
                nc.scalar.activation(
                    gsq[:], psg[:], mybir.ActivationFunctionType.Copy
                )

            def b2_corr():
                for mh in range(mt):
                    for nh in range(nh_n):
                        nc.tensor.matmul(
                            psm[mh][nh][:],
                            gsq[:, 128 * mh : 128 * (mh + 1)],
                            sneg[:, nsl * nh :][:, :nsl],
                            start=False, stop=True,
                        )

            for sti in range(st_n):
                phase_b(sti, last_st=False)
                if sti == 0:
                    # group-sum chain early: by now all qp planes are built,
                    # and the serial PE<->ACT chain overlaps the main stream
                    b2_chain()
            b2_corr()

            # ---- phase C: scale by 1/s and store ----
            for mh in range(mt):
                for nh in range(nh_n):
                    ob = obp.tile([128, nsl], BF16, tag="ob")
                    nc.scalar.activation(
                        ob[:], psm[mh][nh][:],
                        mybir.ActivationFunctionType.Copy, scale=r1s[mh][:],
                    )
                    nc.sync.dma_start(
                        out_r[mh, :, nsl * nh : nsl * (nh + 1)], ob[:]
                    )
    nc.compile()
    return nc


def host_prep(input, weight_scale, weight, ns):
    """Shard + relayout inputs for each core. Pure relayout of static weight
    data (transpose, uint8->uint16 view, group-scale expansion) plus fp32
    activation passthrough."""
    n = weight.shape[0]
    x = np.ascontiguousarray(input, dtype=np.float32)
    w_bytes = weight.astype(np.uint8)              # [N, K/4] packed bytes
    w16 = w_bytes.view(np.uint16)                  # [N, K/8] 8 codes each
    ws2 = np.asarray(weight_scale, dtype=np.float32).reshape(n, -1)  # [N, K/GS]
    ws2_b = ws2.astype(ml_dtypes.bfloat16)
    # gmat[kh, 8*t+gl] = 4^t * (kh//16 == gl)   (t=7 stays unscaled: its
    # qp plane was evacuated with scale 1, matching extraction's 4^0)
    gmat = np.zeros((128, 64), dtype=np.float32)
    for t in range(8):
        for khp in range(128):
            gmat[khp, 8 * t + khp // 16] = 1.0
    gmat = gmat.astype(ml_dtypes.bfloat16)
    in_maps = []
    for c in range(n // ns):
        sl = slice(c * ns, (c + 1) * ns)
        w16_c = np.ascontiguousarray(w16[sl].T)    # [KH, ns]
        se_c = np.ascontiguousarray(ws2_b[sl].T.repeat(16, axis=0))  # [KH, ns]
        sn_c = -np.ascontiguousarray(ws2_b[sl].T).astype(np.float32)  # [K/GS, ns]
        in_maps.append(
            {"x": x, "w16": w16_c, "sexp": se_c, "sneg": sn_c, "gmat": gmat}
        )
    return in_maps


_NC_CACHE = {}


def _get_nc(m, k, ns):
    key = (m, k, ns)
    if key not in _NC_CACHE:
        _NC_CACHE[key] = build_nc(m, k, ns)
    return _NC_CACHE[key]


def kernel(input, weight_scale, weight, group_size=GS, trace=False):
    m, k = input.shape
    n = weight.shape[0]
    ns = n // NCORES
    nc = _get_nc(m, k, ns)
    in_maps = host_prep(input, weight_scale, weight, ns)
    res = bass_utils.run_bass_kernel_spmd(
        nc, in_maps, core_ids=list(range(NCORES)), trace=trace
    )
    out = np.concatenate([r["out"] for r in res.results], axis=1)
    if trace:
        return out, res
    return out


if __name__ == "__main__":
    # small-config CoreSim check
    from concourse.bass_interp import CoreSim

    rng = np.random.default_rng(0)
    m, k, ns = 256, 2048, 256
    x = rng.standard_normal((m, k), dtype=np.float32)
    w_tern = rng.integers(-1, 2, size=(ns, k)).astype(np.int32)
    codes = (w_tern + 1).reshape(ns, k // 4, 4)
    packed = (
        codes[..., 0] | (codes[..., 1] << 2) | (codes[..., 2] << 4)
        | (codes[..., 3] << 6)
    ).astype(np.int32)
    ws = rng.uniform(0.001, 0.02, size=(ns, k // GS, 1)).astype(np.float32)

    # numpy reference
    s = 127.0 / np.clip(np.abs(x).max(axis=-1, keepdims=True), 1e-5, None)
    q = np.clip(np.round(x * s), -128, 127)
    wf = w_tern.astype(np.float32) * np.repeat(ws.reshape(ns, -1), GS, axis=1)
    ref = ((q @ wf.T) / s).astype(ml_dtypes.bfloat16).astype(np.float32)

    nc = build_nc(m, k, ns)
    im = host_prep(x, ws, packed, ns)[0]
    sim = CoreSim(nc)
    for kk, v in im.items():
        sim.tensor(kk)[:] = v
    sim.simulate()
    got = np.asarray(sim.tensor("out")).astype(np.float32)
    err = np.abs(got - ref).max() / (np.abs(ref).max() + 1e-9)
    print("rel err (absmax):", err)
    rms = np.sqrt(((got - ref) ** 2).mean()) / (np.sqrt((ref**2).mean()) + 1e-9)
    print("rel err (rms):", rms)



# revision 2
# speedup vs baseline: 1.0008x; 1.0008x over previous
"""BitLinear (ternary 2-bit weights, group-128 scales, dynamic int8 activation
quant) for Trainium2, tensor-parallel over 8 NeuronCores (shard N).

Math (per core, N-shard ns):
  s[m]   = 127 / clip(max_k |x[m,k]|, 1e-5)
  q[m,k] = round(x[m,k] * s[m])                      (integers in [-127,127])
  out[m,n] = (sum_k q[m,k] * w[n,k] * ws[n, k//128]) / s[m]   -> bf16

Device scheme ("stream-W"): the ternary weight is decoded host-side into the
full bf16 W[k, n] = (code-1) * ws (exact: +-1 times a bf16 scale is exact),
and streamed tile-by-tile from HBM while the PE consumes it.  This removes
the on-device 2-bit decode entirely (DVE extract+multiply was ~59us of
serial DVE work, rivaling the PE as the bottleneck) at the cost of 16MB of
weight DMA per core -- still comfortably under the PE-bound span.

Pipeline per core:
  A: DMA x in [128, 2048] chunks; DVE rolling row-abs-max; s = 127*recip(max).
  B: ACT quant: t1 = x*s + 2^23 (RNE via magic number), q = t1 - 2^23 (bf16).
  C: PE transposes q chunks into qT[k, m] via identity matmuls
     (4 per PSUM bank, evacuated fp32->bf16 on ACT/DVE alternating).
  D: PE mains: psm[mh][nh] += qT-slice.T @ W-tile, accumulating over all 64
     k-blocks; W tiles stream from DRAM in 1MB units.
  E: evac psm * (rowmax/127) -> bf16 -> DMA out.

Dummy matmuls tied to x-chunk arrivals keep the PE HAM clock at 2.4 GHz
before the real matmul stream begins.
"""

import sys

import numpy as np

try:
    import concourse.bass as bass
except ImportError:  # fresh grading dir: fall back to the repo checkout
    sys.path.insert(0, "/opt/trn_rl_repo")
    import concourse.bass as bass

import ml_dtypes

import concourse.mybir as mybir
import concourse.tile as tile
from concourse import bacc, bass_utils
from concourse.masks import make_identity

FP32 = mybir.dt.float32
BF16 = mybir.dt.bfloat16
MAGIC = float(2 << 22)  # 2^23

M, N, K, GS = 256, 8192, 8192, 128
NCORES = 8


def build_nc(m=M, k=K, ns=N // NCORES):
    """One core's program: full m,k; n-shard of size ns."""
    mt = m // 128        # m partition-tiles
    ck = k // 2048       # 2048-wide k-chunks for quant
    kb = k // 128        # k-blocks (contraction tiles)
    wu = kb // 4         # W DMA units of 4 k-blocks (1MB each)
    nsl = min(512, ns)   # matmul rhs free-dim slice (PSUM bank width)
    nh_n = ns // nsl

    nc = bacc.Bacc()
    x_d = nc.declare_dram_parameter("x", [m, k], FP32, isOutput=False)
    w_d = nc.declare_dram_parameter("wf", [k, ns], BF16, isOutput=False)
    out_d = nc.declare_dram_parameter("out", [m, ns], BF16, isOutput=True)

    x_r = x_d.rearrange("(T p) k -> T p k", p=128)            # [mt,128,k]
    w_r = w_d.rearrange("(u f p) n -> u p f n", f=4, p=128)   # [wu,128,4,ns]
    out_r = out_d.rearrange("(T p) n -> T p n", p=128)        # [mt,128,ns]

    with tile.TileContext(nc) as tc:
        with (
            tc.tile_pool(name="const", bufs=1) as constp,
            tc.tile_pool(name="stat", bufs=1) as statp,
            tc.tile_pool(name="qc", bufs=1) as qcp,
            tc.tile_pool(name="qt", bufs=1) as qtp,
            tc.tile_pool(name="xp", bufs=4) as xp,
            tc.tile_pool(name="t1", bufs=2) as t1p,
            tc.tile_pool(name="wp", bufs=4) as wpool,
            tc.tile_pool(name="ob", bufs=4) as obp,
            tc.tile_pool(name="psx", bufs=1, space="PSUM") as psxp,
            tc.tile_pool(name="pst", bufs=2, space="PSUM") as pstp,
            tc.tile_pool(name="psm", bufs=1, space="PSUM") as psmp,
        ):
            ident = constp.tile([128, 128], BF16, tag="ident")
            make_identity(nc, ident)

            def warm(dep_tile_128xN, width, n_mm=1, name=""):
                """Dummy matmuls reading an already-landed tile: keep the PE
                HAM activity monitor from re-throttling to 1.2 GHz."""
                for j in range(n_mm):
                    wp_ = psxp.tile([128, 512], FP32, tag="psx",
                                    name=f"wrm{name}{j}")
                    nc.tensor.matmul(
                        wp_[:, :width], dep_tile_128xN[:, :128],
                        dep_tile_128xN[:, :width],
                        start=True, stop=True,
                    )

            # persistent activation buffers
            qc = [qcp.tile([128, k], BF16, tag=f"qc{t}", name=f"qc{t}")
                  for t in range(mt)]
            # qT[kb][p, m]: transposed q, bf16, [128, kb, m]
            qt = qtp.tile([128, kb * m], BF16, tag="qt")
            qt3 = qt.rearrange("p (B mm) -> p B mm", B=kb)

            rpart = [statp.tile([128, ck], FP32, tag=f"rp{t}", name=f"rp{t}")
                     for t in range(mt)]
            rmax = [statp.tile([128, 1], FP32, tag=f"rm{t}", name=f"rm{t}")
                    for t in range(mt)]
            s_pp = [statp.tile([128, 1], FP32, tag=f"sp{t}", name=f"sp{t}")
                    for t in range(mt)]
            r1s = [statp.tile([128, 1], FP32, tag=f"rs{t}", name=f"rs{t}")
                   for t in range(mt)]

            psm = [
                [psmp.tile([128, nsl], FP32, tag=f"ps{mh}{nh}",
                           name=f"ps{mh}{nh}")
                 for nh in range(nh_n)]
                for mh in range(mt)
            ]

            # ---- W stream: issue all weight-tile DMAs up front; the Tile
            # runtime paces them against pool buffer reuse (bufs=4) ----
            wtiles = {}

            def load_w(u):
                wt = wpool.tile([128, 4 * ns], BF16, tag="wt", name=f"wt{u}")
                wt3 = wt.rearrange("p (f n) -> p f n", f=4)
                nc.sync.dma_start(wt3[:], w_r[u])
                wtiles[u] = wt3

            # ---- phase A+B+C per m-tile: load, rowmax, quant, transpose ----
            def quant_mtile(mh):
                xs = []
                for c in range(ck):
                    sl = slice(2048 * c, 2048 * (c + 1))
                    xc = xp.tile([128, 2048], FP32, tag="x", name=f"x{mh}{c}")
                    nc.sync.dma_start(xc[:], x_r[mh, :, sl])
                    nc.vector.tensor_reduce(
                        rpart[mh][:, c : c + 1], xc[:],
                        axis=mybir.AxisListType.X, op=mybir.AluOpType.max,
                        apply_absolute_value=True,
                    )
                    xs.append(xc)
                    # PE keep-warm tied to chunk arrival
                    if mh == 0:
                        warm(xc, 512, n_mm=2 if c < ck - 1 else 5,
                             name=f"a{c}")
                nc.vector.tensor_reduce(
                    rmax[mh][:], rpart[mh][:],
                    axis=mybir.AxisListType.X, op=mybir.AluOpType.max,
                )
                nc.vector.tensor_scalar_max(rmax[mh][:], rmax[mh][:], 1e-5)
                nc.vector.reciprocal(s_pp[mh][:], rmax[mh][:])
                nc.vector.tensor_scalar_mul(s_pp[mh][:], s_pp[mh][:], 127.0)
                nc.vector.tensor_scalar_mul(r1s[mh][:], rmax[mh][:],
                                            1.0 / 127.0)
                for c in range(ck):
                    sl = slice(2048 * c, 2048 * (c + 1))
                    t1 = t1p.tile([128, 2048], FP32, tag="t1")
                    # fma(x, s, 2^23) - 2^23 rounds x*s to nearest (RNE),
                    # matching jnp.round; both passes on ACT.
                    nc.scalar.activation(
                        t1[:], xs[c][:],
                        mybir.ActivationFunctionType.Copy,
                        bias=MAGIC, scale=s_pp[mh][:],
                    )
                    nc.scalar.activation(
                        qc[mh][:, sl], t1[:],
                        mybir.ActivationFunctionType.Copy, bias=-MAGIC,
                    )

            evac_i = 0

            def transpose_quad(mh, b0):
                """Transpose q k-blocks b0..b0+3 of m-tile mh into qt."""
                nonlocal evac_i
                psT = pstp.tile([128, 512], FP32, tag="psT")
                for j in range(4):
                    nc.tensor.matmul(
                        psT[:, 128 * j : 128 * (j + 1)],
                        qc[mh][:, 128 * (b0 + j) :][:, :128], ident[:],
                        start=True, stop=True,
                    )
                # evac fp32->bf16 into qt[b0..b0+3, mh*128:...]
                dst = qt3[:, b0 : b0 + 4, 128 * mh : 128 * (mh + 1)]
                if evac_i % 2 == 0:
                    nc.scalar.activation(
                        dst, psT[:], mybir.ActivationFunctionType.Copy
                    )
                else:
                    nc.vector.tensor_copy(dst, psT[:])
                evac_i += 1

            def mains(mh, b):
                u, f = b // 4, b % 4
                wt3 = wtiles[u]
                lhsT = qt3[:, b, 128 * mh : 128 * (mh + 1)]
                for nh in range(nh_n):
                    nc.tensor.matmul(
                        psm[mh][nh][:],
                        lhsT,
                        wt3[:, f, nsl * nh : nsl * (nh + 1)],
                        start=(b == 0), stop=(b == kb - 1),
                    )

            # ---------------- emission schedule ----------------
            # m-tile 0: x DMA + rowmax + quant
            quant_mtile(0)
            # W stream starts (x mt0 already queued ahead of it)
            for u in range(4):
                load_w(u)
            # mt0 transposes + early mains (mh=0)
            for b0 in range(0, 16, 4):
                transpose_quad(0, b0)
            # m-tile 1 x DMA + quant queued; W continues behind it
            quant_mtile(1)
            for u in range(4, wu):
                load_w(u)
            for b0 in range(16, kb, 4):
                transpose_quad(0, b0)
            for b in range(0, 12):
                mains(0, b)
            for b0 in range(0, kb, 4):
                transpose_quad(1, b0)
            # paired mains: mh0 catch-up + mh1
            for b in range(0, 12):
                mains(1, b)
            for b in range(12, kb):
                mains(0, b)
                mains(1, b)

            # ---- final: scale by rowmax/127 and store ----
            for mh in range(mt):
                for nh in range(nh_n):
                    ob = obp.tile([128, nsl], BF16, tag="ob")
                    nc.scalar.activation(
                        ob[:], psm[mh][nh][:],
                        mybir.ActivationFunctionType.Copy, scale=r1s[mh][:],
                    )
                    nc.sync.dma_start(
                        out_r[mh, :, nsl * nh : nsl * (nh + 1)], ob[:]
                    )
    nc.compile()
    return nc


def host_prep(input, weight_scale, weight, ns):
    """Shard + relayout inputs for each core: decode the packed 2-bit ternary
    codes and fold the per-(row, group) scale into a full bf16 W[k, n] per
    core (pure static-weight relayout), plus fp32 activation passthrough."""
    n, k4 = weight.shape
    k = k4 * 4
    x = np.ascontiguousarray(input, dtype=np.float32)
    w_bytes = weight.astype(np.uint8)                       # [N, K/4]
    codes = np.empty((n, k), dtype=np.int8)                 # [N, K] in {-1,0,1}
    for j in range(4):
        codes[:, j::4] = ((w_bytes >> (2 * j)) & 3).astype(np.int8) - 1
    ws2 = np.asarray(weight_scale, dtype=np.float32).reshape(n, -1)  # [N, K/GS]
    ws2_b = ws2.astype(ml_dtypes.bfloat16)
    # W[n, k] = codes * ws (exact in bf16: +-1 * bf16 scale)
    wf = codes.astype(np.float32) * ws2_b.astype(np.float32).repeat(GS, axis=1)
    wf = wf.astype(ml_dtypes.bfloat16)
    in_maps = []
    for c in range(n // ns):
        sl = slice(c * ns, (c + 1) * ns)
        wf_c = np.ascontiguousarray(wf[sl].T)               # [K, ns] bf16
        in_maps.append({"x": x, "wf": wf_c})
    return in_maps


_NC_CACHE = {}


def _get_nc(m, k, ns):
    key = (m, k, ns)
    if key not in _NC_CACHE:
        _NC_CACHE[key] = build_nc(m, k, ns)
    return _NC_CACHE[key]


def kernel(input, weight_scale, weight, group_size=GS, trace=False):
    m, k = input.shape
    n = weight.shape[0]
    ns = n // NCORES
    nc = _get_nc(m, k, ns)
    in_maps = host_prep(input, weight_scale, weight, ns)
    res = bass_utils.run_bass_kernel_spmd(
        nc, in_maps, core_ids=list(range(NCORES)), trace=trace
    )
    out = np.concatenate([r["out"] for r in res.results], axis=1)
    if trace:
        return out, res
    return out


if __name__ == "__main__":
    # small-config CoreSim check
    from concourse.bass_interp import CoreSim

    rng = np.random.default_rng(0)
    m, k, ns = 256, 2048, 256
    x = rng.standard_normal((m, k), dtype=np.float32)
    w_tern = rng.integers(-1, 2, size=(ns, k)).astype(np.int32)
    codes = (w_tern + 1).reshape(ns, k // 4, 4)
    packed = (
        codes[..., 0] | (codes[..., 1] << 2) | (codes[..., 2] << 4)
        | (codes[..., 3] << 6)
    ).astype(np.int32)
    ws = rng.uniform(0.001, 0.02, size=(ns, k // GS, 1)).astype(np.float32)

    # numpy reference
    s = 127.0 / np.clip(np.abs(x).max(axis=-1, keepdims=True), 1e-5, None)
    q = np.clip(np.round(x * s), -128, 127)
    wf = w_tern.astype(np.float32) * np.repeat(ws.reshape(ns, -1), GS, axis=1)
    ref = ((q @ wf.T) / s).astype(ml_dtypes.bfloat16).astype(np.float32)

    nc = build_nc(m, k, ns)
    im = host_prep(x, ws, packed, ns)[0]
    sim = CoreSim(nc)
    for kk, v in im.items():
        sim.tensor(kk)[:] = v
    sim.simulate()
    got = np.asarray(sim.tensor("out")).astype(np.float32)
    err = np.abs(got - ref).max() / (np.abs(ref).max() + 1e-9)
    print("rel err (absmax):", err)
    rms = np.sqrt(((got - ref) ** 2).mean()) / (np.sqrt((ref**2).mean()) + 1e-9)
    print("rel err (rms):", rms)


# revision 6
# speedup vs baseline: 1.0180x; 1.0172x over previous
"""BitLinear (ternary 2-bit weights, group-128 scales, dynamic int8 activation
quant) for Trainium2, tensor-parallel over 8 NeuronCores (shard N).

Math (per core, N-shard ns):
  s[m]   = 127 / clip(max_k |x[m,k]|, 1e-5)
  q[m,k] = round(x[m,k] * s[m])                      (integers in [-127,127])
  out[m,n] = (sum_k q[m,k] * w[n,k] * ws[n, k//128]) / s[m]   -> bf16

Device scheme ("stream-W"): the ternary weight is decoded host-side into the
full bf16 W[k, n] = (code-1) * ws (exact: +-1 times a bf16 scale is exact),
and streamed tile-by-tile from HBM while the PE consumes it.  This removes
the on-device 2-bit decode entirely (DVE extract+multiply was ~59us of
serial DVE work, rivaling the PE as the bottleneck) at the cost of 16MB of
weight DMA per core -- still under the PE-bound span.

Schedule: x-mt0 loads first at full HBM bandwidth (weight DMAs are gated on
the mt0 rowmax via dummy-write deps so they cannot compete); the first 6
weight units (k-blocks 0-23) live in a resident pool, consumed by mh0 mains
early (P1, while mt1 still quantizes) and re-read by mh1 mains at the very
end (P3); units 6-15 stream through a 4-buffer pool consumed by paired
(mh0+mh1) mains (P2).  PSUM accumulation order per psm tile is therefore
non-monotonic in k, which is fine (start/stop flags set by emission order).
Dummy matmuls tied to x-chunk arrivals keep the PE HAM clock ramped.
"""

import sys

import numpy as np

try:
    import concourse.bass as bass
except ImportError:  # fresh grading dir: fall back to the repo checkout
    sys.path.insert(0, "/opt/trn_rl_repo")
    import concourse.bass as bass

import ml_dtypes

import concourse.mybir as mybir
import concourse.tile as tile
from concourse import bacc, bass_utils
from concourse.masks import make_identity

FP32 = mybir.dt.float32
BF16 = mybir.dt.bfloat16
MAGIC = float(2 << 22)  # 2^23

M, N, K, GS = 256, 8192, 8192, 128
NCORES = 8


def build_nc(m=M, k=K, ns=N // NCORES):
    """One core's program: full m,k; n-shard of size ns."""
    mt = m // 128        # m partition-tiles (2)
    ck = k // 2048       # 2048-wide k-chunks for quant (4)
    kb = k // 128        # k-blocks / contraction tiles (64)
    nq = kb // 4         # transpose quads (16)
    wu = kb // 4         # W DMA units of 4 k-blocks, 1MB each (16)
    n_res = min(6, wu)   # resident W units (k-blocks 0 .. 4*n_res-1)
    bres = 4 * n_res
    nsl = min(512, ns)   # matmul rhs free-dim slice (PSUM bank width)
    nh_n = ns // nsl

    nc = bacc.Bacc()
    x_d = nc.declare_dram_parameter("x", [m, k], FP32, isOutput=False)
    w_d = nc.declare_dram_parameter("wf", [k, ns], BF16, isOutput=False)
    out_d = nc.declare_dram_parameter("out", [m, ns], BF16, isOutput=True)

    x_r = x_d.rearrange("(T p) k -> T p k", p=128)            # [mt,128,k]
    w_r = w_d.rearrange("(u f p) n -> u p f n", f=4, p=128)   # [wu,128,4,ns]
    out_r = out_d.rearrange("(T p) n -> T p n", p=128)        # [mt,128,ns]

    with tile.TileContext(nc) as tc:
        with (
            tc.tile_pool(name="const", bufs=1) as constp,
            tc.tile_pool(name="stat", bufs=1) as statp,
            tc.tile_pool(name="qcc", bufs=3) as qccp,
            tc.tile_pool(name="qt", bufs=1) as qtp,
            tc.tile_pool(name="xp", bufs=4) as xp,
            tc.tile_pool(name="t1", bufs=2) as t1p,
            tc.tile_pool(name="wres", bufs=1) as wresp,
            tc.tile_pool(name="wstr", bufs=4) as wstrp,
            tc.tile_pool(name="ob", bufs=4) as obp,
            tc.tile_pool(name="psx", bufs=1, space="PSUM") as psxp,
            tc.tile_pool(name="pst", bufs=2, space="PSUM") as pstp,
            tc.tile_pool(name="psm", bufs=1, space="PSUM") as psmp,
        ):
            ident = constp.tile([128, 128], BF16, tag="ident")
            make_identity(nc, ident)

            def warm(dep_tile, width, n_mm=1, name=""):
                """Dummy matmuls reading an already-landed tile: keep the PE
                HAM activity monitor from throttling before the real stream."""
                for j in range(n_mm):
                    wp_ = psxp.tile([128, 512], FP32, tag="psx",
                                    name=f"wrm{name}{j}")
                    nc.tensor.matmul(
                        wp_[:, :width], dep_tile[:, :128],
                        dep_tile[:, :width],
                        start=True, stop=True,
                    )

            # quantized activations, one tile per (mh, chunk)
            qcs = {}
            # transposed q, one tile per (mh, quad): [128, 4 kb-sub, 128 m]
            qtq = {
                (mh, qd): qtp.tile([128, 512], BF16, tag=f"qt{mh}_{qd}",
                                   name=f"qt{mh}_{qd}")
                for mh in range(mt) for qd in range(nq)
            }

            rpart = [statp.tile([128, ck], FP32, tag=f"rp{t}", name=f"rp{t}")
                     for t in range(mt)]
            rmax = [statp.tile([128, 1], FP32, tag=f"rm{t}", name=f"rm{t}")
                    for t in range(mt)]
            s_pp = [statp.tile([128, 1], FP32, tag=f"sp{t}", name=f"sp{t}")
                    for t in range(mt)]
            r1s = [statp.tile([128, 1], FP32, tag=f"rs{t}", name=f"rs{t}")
                   for t in range(mt)]

            psm = [
                [psmp.tile([128, nsl], FP32, tag=f"ps{mh}{nh}",
                           name=f"ps{mh}{nh}")
                 for nh in range(nh_n)]
                for mh in range(mt)
            ]

            wtiles = {}

            def load_w(u, pool, gate=None):
                wt = pool.tile([128, 4 * ns], BF16,
                               tag="wt" if pool is wstrp else f"wr{u}",
                               name=f"w{u}")
                if gate is not None:
                    # dummy write reading `gate`: delays this DMA until gate
                    # is produced (keeps W off the HBM while x streams)
                    nc.gpsimd.tensor_copy(wt[:1, :1], gate[:1, :1])
                wt3 = wt.rearrange("p (f n) -> p f n", f=4)
                nc.sync.dma_start(wt3[:], w_r[u])
                wtiles[u] = wt3

            def load_x_rowmax(mh):
                xs = []
                for c in range(ck):
                    sl = slice(2048 * c, 2048 * (c + 1))
                    xc = xp.tile([128, 2048], FP32, tag="x", name=f"x{mh}{c}")
                    nc.sync.dma_start(xc[:], x_r[mh, :, sl])
                    nc.vector.tensor_reduce(
                        rpart[mh][:, c : c + 1], xc[:],
                        axis=mybir.AxisListType.X, op=mybir.AluOpType.max,
                        apply_absolute_value=True,
                    )
                    xs.append(xc)
                    if mh == 0:
                        warm(xc, 512, n_mm=2 if c < ck - 1 else 5,
                             name=f"a{c}")
                nc.vector.tensor_reduce(
                    rmax[mh][:], rpart[mh][:],
                    axis=mybir.AxisListType.X, op=mybir.AluOpType.max,
                )
                nc.vector.tensor_scalar_max(rmax[mh][:], rmax[mh][:], 1e-5)
                nc.vector.reciprocal(s_pp[mh][:], rmax[mh][:])
                nc.vector.tensor_scalar_mul(s_pp[mh][:], s_pp[mh][:], 127.0)
                nc.vector.tensor_scalar_mul(r1s[mh][:], rmax[mh][:],
                                            1.0 / 127.0)
                return xs

            def quant_chunk(mh, c, xc):
                t1 = t1p.tile([128, 2048], FP32, tag="t1")
                # fma(x, s, 2^23) - 2^23 rounds x*s to nearest (RNE),
                # matching jnp.round; both passes on ACT.
                nc.scalar.activation(
                    t1[:], xc[:], mybir.ActivationFunctionType.Copy,
                    bias=MAGIC, scale=s_pp[mh][:],
                )
                qc = qccp.tile([128, 2048], BF16, tag="qc", name=f"qc{mh}{c}")
                nc.scalar.activation(
                    qc[:], t1[:], mybir.ActivationFunctionType.Copy,
                    bias=-MAGIC,
                )
                qcs[(mh, c)] = qc

            def transpose_quad(mh, qd, evac_eng):
                """Transpose k-blocks 4qd..4qd+3 of m-tile mh into qtq."""
                qc = qcs[(mh, qd // (nq // ck))]
                psT = pstp.tile([128, 512], FP32, tag="psT")
                for j in range(4):
                    off = 128 * ((4 * qd + j) % 16)
                    nc.tensor.matmul(
                        psT[:, 128 * j : 128 * (j + 1)],
                        qc[:, off : off + 128], ident[:],
                        start=True, stop=True,
                    )
                dst = qtq[(mh, qd)][:]
                if evac_eng == 0:
                    nc.scalar.activation(
                        dst, psT[:], mybir.ActivationFunctionType.Copy
                    )
                else:
                    nc.vector.tensor_copy(dst, psT[:])

            started = [[False] * nh_n for _ in range(mt)]

            def mains(mh, b, stop=False):
                u = b // 4
                wt3 = wtiles[u]
                lhsT = qtq[(mh, b // 4)].rearrange(
                    "p (f mm) -> p f mm", f=4)[:, b % 4, :]
                for nh in range(nh_n):
                    nc.tensor.matmul(
                        psm[mh][nh][:],
                        lhsT,
                        wt3[:, b % 4, nsl * nh : nsl * (nh + 1)],
                        start=not started[mh][nh], stop=stop,
                    )
                    started[mh][nh] = True

            def finalize(mh):
                for nh in range(nh_n):
                    ob = obp.tile([128, nsl], BF16, tag="ob")
                    nc.scalar.activation(
                        ob[:], psm[mh][nh][:],
                        mybir.ActivationFunctionType.Copy, scale=r1s[mh][:],
                    )
                    nc.scalar.dma_start(
                        out_r[mh, :, nsl * nh : nsl * (nh + 1)], ob[:]
                    )

            # ---------------- emission schedule ----------------
            qpc = nq // ck          # quads per chunk (4)
            xs0 = load_x_rowmax(0)
            # W residents: first two units gated on mt0 rowmax (x priority)
            for u in range(min(2, n_res)):
                load_w(u, wresp, gate=rmax[0])
            # mt1 x DMAs: naturally paced by xp buffer reuse (after mt0 pass1)
            xs1 = load_x_rowmax(1)
            for u in range(2, n_res):
                load_w(u, wresp)
            # mt0 quant + transposes (evacs on ACT/DVE alternating)
            for c in range(ck):
                quant_chunk(0, c, xs0[c])
                for j in range(qpc):
                    transpose_quad(0, c * qpc + j, evac_eng=j % 2)
            # stream W: first window gated on mt1 rowmax
            for u in range(n_res, min(n_res + 4, wu)):
                load_w(u, wstrp, gate=rmax[1])
            # P1: early mh0 mains on resident W
            for b in range(0, bres):
                mains(0, b, stop=(wu == n_res and b == kb - 1))
            # mt1 quant + transposes (evacs on DVE/ACT alternating)
            for c in range(ck):
                quant_chunk(1, c, xs1[c])
                for j in range(qpc):
                    transpose_quad(1, c * qpc + j, evac_eng=(j + 1) % 2)
            for u in range(n_res + 4, wu):
                load_w(u, wstrp)
            # P2: paired mains on streamed W
            for b in range(bres, kb):
                mains(0, b, stop=(b == kb - 1))
                mains(1, b)
            finalize(0)
            # P3: mh1 catch-up on resident W
            for b in range(0, bres):
                mains(1, b, stop=(b == bres - 1))
            finalize(1)
    nc.compile()
    return nc


def host_prep(input, weight_scale, weight, ns):
    """Shard + relayout inputs for each core: decode the packed 2-bit ternary
    codes and fold the per-(row, group) scale into a full bf16 W[k, n] per
    core (pure static-weight relayout), plus fp32 activation passthrough."""
    n, k4 = weight.shape
    k = k4 * 4
    x = np.ascontiguousarray(input, dtype=np.float32)
    w_bytes = weight.astype(np.uint8)                       # [N, K/4]
    codes = np.empty((n, k), dtype=np.int8)                 # [N, K] in {-1,0,1}
    for j in range(4):
        codes[:, j::4] = ((w_bytes >> (2 * j)) & 3).astype(np.int8) - 1
    ws2 = np.asarray(weight_scale, dtype=np.float32).reshape(n, -1)  # [N, K/GS]
    ws2_b = ws2.astype(ml_dtypes.bfloat16)
    # W[n, k] = codes * ws (exact in bf16: +-1 * bf16 scale)
    wf = codes.astype(np.float32) * ws2_b.astype(np.float32).repeat(GS, axis=1)
    wf = wf.astype(ml_dtypes.bfloat16)
    in_maps = []
    for c in range(n // ns):
        sl = slice(c * ns, (c + 1) * ns)
        wf_c = np.ascontiguousarray(wf[sl].T)               # [K, ns] bf16
        in_maps.append({"x": x, "wf": wf_c})
    return in_maps


_NC_CACHE = {}


def _get_nc(m, k, ns):
    key = (m, k, ns)
    if key not in _NC_CACHE:
        _NC_CACHE[key] = build_nc(m, k, ns)
    return _NC_CACHE[key]


def kernel(input, weight_scale, weight, group_size=GS, trace=False):
    m, k = input.shape
    n = weight.shape[0]
    ns = n // NCORES
    nc = _get_nc(m, k, ns)
    in_maps = host_prep(input, weight_scale, weight, ns)
    res = bass_utils.run_bass_kernel_spmd(
        nc, in_maps, core_ids=list(range(NCORES)), trace=trace
    )
    out = np.concatenate([r["out"] for r in res.results], axis=1)
    if trace:
        return out, res
    return out


if __name__ == "__main__":
    # small-config CoreSim check
    from concourse.bass_interp import CoreSim

    rng = np.random.default_rng(0)
    m, k, ns = 256, 2048, 256
    x = rng.standard_normal((m, k), dtype=np.float32)
    w_tern = rng.integers(-1, 2, size=(ns, k)).astype(np.int32)
    codes = (w_tern + 1).reshape(ns, k // 4, 4)
    packed = (
        codes[..., 0] | (codes[..., 1] << 2) | (codes[..., 2] << 4)
        | (codes[..., 3] << 6)
    ).astype(np.int32)
    ws = rng.uniform(0.001, 0.02, size=(ns, k // GS, 1)).astype(np.float32)

    # numpy reference
    s = 127.0 / np.clip(np.abs(x).max(axis=-1, keepdims=True), 1e-5, None)
    q = np.clip(np.round(x * s), -128, 127)
    wf = w_tern.astype(np.float32) * np.repeat(ws.reshape(ns, -1), GS, axis=1)
    ref = ((q @ wf.T) / s).astype(ml_dtypes.bfloat16).astype(np.float32)

    nc = build_nc(m, k, ns)
    im = host_prep(x, ws, packed, ns)[0]
    sim = CoreSim(nc)
    for kk, v in im.items():
        sim.tensor(kk)[:] = v
    sim.simulate()
    got = np.asarray(sim.tensor("out")).astype(np.float32)
    err = np.abs(got - ref).max() / (np.abs(ref).max() + 1e-9)
    print("rel err (absmax):", err)
    rms = np.sqrt(((got - ref) ** 2).mean()) / (np.sqrt((ref**2).mean()) + 1e-9)
    print("rel err (rms):", rms)


# revision 11
# speedup vs baseline: 1.1852x; 1.1642x over previous
"""BitLinear (ternary 2-bit weights, group-128 scales, dynamic int8 activation
quant) for Trainium2, tensor-parallel over 8 NeuronCores (shard N).

Math (per core, N-shard ns):
  s[m]   = 127 / clip(max_k |x[m,k]|, 1e-5)
  q[m,k] = round(x[m,k] * s[m])                      (integers in [-127,127])
  out[m,n] = (sum_k q[m,k] * w[n,k] * ws[n, k//128]) / s[m]   -> bf16

Device scheme ("stream-W" + int16 transposes):
- W is decoded host-side to full bf16 W[k,n] = (code-1)*ws (exact) and
  streamed from HBM; no on-device decode at all.
- Quantization is ONE activation pass: t1 = x*s + 2^23 (RNE rounding via the
  fp32 magic number).  t1's low 16 bits ARE q as int16 two's complement, so
  the PE transposes t1.bitcast(int16) stride-2 views directly
  (is_transpose=True, 1 cycle/row for 16-bit dtypes); the PSUM->SBUF evac
  converts int16 -> bf16.  The explicit "subtract 2^23" pass disappears.
- Mains: psm[mh][nh] += qT-slice.T @ W-tile over 64 k-blocks.
- Schedule: x first at full HBM bw (W gated on rowmax progress via dummy
  writes); resident W units (k-blocks 0..15) serve early mh0 mains (P1) and
  mh1 catch-up (P3, before P2 so both psm tiles finish together); units 4-15
  stream through a rotating pool for paired mains (P2).
- Dense fp32 dummy-matmul blocks tied to x-chunk arrivals keep the PE DVFS
  ramped through the head phase.
"""

import sys

import numpy as np

try:
    import concourse.bass as bass
except ImportError:  # fresh grading dir: fall back to the repo checkout
    sys.path.insert(0, "/opt/trn_rl_repo")
    import concourse.bass as bass

import ml_dtypes

import concourse.mybir as mybir
import concourse.tile as tile
from concourse import bacc, bass_utils
from concourse.masks import make_identity

FP32 = mybir.dt.float32
BF16 = mybir.dt.bfloat16
I16 = mybir.dt.int16
F16 = mybir.dt.float16
# 1.5 * 2^23: fp32 RNE rounds x*s to an integer AND every value in
# [1.5*2^23 - 128, 1.5*2^23 + 127] keeps exponent 150, so the low 16
# mantissa bits are exactly q in two's complement.
MAGIC = float(3 << 22)

M, N, K, GS = 256, 8192, 8192, 128
NCORES = 8


def build_nc(m=M, k=K, ns=N // NCORES):
    """One core's program: full m,k; n-shard of size ns."""
    mt = m // 128        # m partition-tiles (2)
    ck = k // 2048       # 2048-wide k-chunks for quant (4)
    kb = k // 128        # k-blocks / contraction tiles (64)
    npr = kb // 8        # transpose pairs: 8 k-blocks per PSUM bank (8)
    wu = kb // 4         # W DMA units of 4 k-blocks, 1MB each (16)
    n_res = min(4, wu)   # resident W units (k-blocks 0 .. 4*n_res-1)
    bres = 4 * n_res
    nsl = min(512, ns)   # matmul rhs free-dim slice (PSUM bank width)
    nh_n = ns // nsl

    nc = bacc.Bacc()
    x_d = nc.declare_dram_parameter("x", [m, k], FP32, isOutput=False)
    w_d = nc.declare_dram_parameter("wf", [k, ns], BF16, isOutput=False)
    out_d = nc.declare_dram_parameter("out", [m, ns], BF16, isOutput=True)

    x_r = x_d.rearrange("(T p) k -> T p k", p=128)            # [mt,128,k]
    w_r = w_d.rearrange("(u f p) n -> u p f n", f=4, p=128)   # [wu,128,4,ns]
    out_r = out_d.rearrange("(T p) n -> T p n", p=128)        # [mt,128,ns]

    with tile.TileContext(nc) as tc:
        with (
            tc.tile_pool(name="const", bufs=1) as constp,
            tc.tile_pool(name="stat", bufs=1) as statp,
            tc.tile_pool(name="qt", bufs=1) as qtp,
            tc.tile_pool(name="xp", bufs=6) as xp,
            tc.tile_pool(name="t1", bufs=5) as t1p,
            tc.tile_pool(name="wres", bufs=1) as wresp,
            tc.tile_pool(name="wstr", bufs=4) as wstrp,
            tc.tile_pool(name="ob", bufs=4) as obp,
            tc.tile_pool(name="psx", bufs=1, space="PSUM") as psxp,
            tc.tile_pool(name="pst", bufs=3, space="PSUM") as pstp,
            tc.tile_pool(name="psm", bufs=1, space="PSUM") as psmp,
        ):
            ident = constp.tile([128, 128], F16, tag="ident")
            make_identity(nc, ident)

            def warm(dep_tile, width, n_mm=1, name=""):
                """Dense fp32 dummy matmuls reading a landed tile: keep the
                PE DVFS/HAM ramped through the head phase."""
                for j in range(n_mm):
                    wp_ = psxp.tile([128, 512], FP32, tag="psx",
                                    name=f"wrm{name}{j}")
                    nc.tensor.matmul(
                        wp_[:, :width], dep_tile[:, :128],
                        dep_tile[:, :width],
                        start=True, stop=True,
                    )

            # t1 tiles (q in the low int16 of each fp32), per (mh, chunk)
            t1s = {}
            # transposed q pairs: [128, 8 kb-sub, 128 m] bf16 per (mh, pair)
            qtqp = {
                (mh, pr): qtp.tile([128, 1024], BF16, tag=f"qt{mh}_{pr}",
                                   name=f"qt{mh}_{pr}")
                for mh in range(mt) for pr in range(npr)
            }

            rpart = [statp.tile([128, ck], FP32, tag=f"rp{t}", name=f"rp{t}")
                     for t in range(mt)]
            rmax = [statp.tile([128, 1], FP32, tag=f"rm{t}", name=f"rm{t}")
                    for t in range(mt)]
            s_pp = [statp.tile([128, 1], FP32, tag=f"sp{t}", name=f"sp{t}")
                    for t in range(mt)]
            r1s = [statp.tile([128, 1], FP32, tag=f"rs{t}", name=f"rs{t}")
                   for t in range(mt)]

            psm = [
                [psmp.tile([128, nsl], FP32, tag=f"ps{mh}{nh}",
                           name=f"ps{mh}{nh}")
                 for nh in range(nh_n)]
                for mh in range(mt)
            ]

            wtiles = {}

            def load_w(u, pool, gate=None):
                wt = pool.tile([128, 4 * ns], BF16,
                               tag="wt" if pool is wstrp else f"wr{u}",
                               name=f"w{u}")
                if gate is not None:
                    # dummy write reading `gate`: delays this DMA until gate
                    # is produced (keeps W off the HBM while x streams)
                    nc.gpsimd.tensor_copy(wt[:1, :1], gate[:1, :1])
                wt3 = wt.rearrange("p (f n) -> p f n", f=4)
                nc.sync.dma_start(wt3[:], w_r[u])
                wtiles[u] = wt3

            def load_x(mh):
                xs = []
                for c in range(ck):
                    sl = slice(2048 * c, 2048 * (c + 1))
                    xc = xp.tile([128, 2048], FP32, tag="x", name=f"x{mh}{c}")
                    nc.sync.dma_start(xc[:], x_r[mh, :, sl])
                    xs.append(xc)
                return xs

            def rowmax(mh, xs):
                for c in range(ck):
                    nc.vector.tensor_reduce(
                        rpart[mh][:, c : c + 1], xs[c][:],
                        axis=mybir.AxisListType.X, op=mybir.AluOpType.max,
                        apply_absolute_value=True,
                    )
                nc.vector.tensor_reduce(
                    rmax[mh][:], rpart[mh][:],
                    axis=mybir.AxisListType.X, op=mybir.AluOpType.max,
                )
                nc.vector.tensor_scalar_max(rmax[mh][:], rmax[mh][:], 1e-5)
                nc.vector.reciprocal(s_pp[mh][:], rmax[mh][:])
                nc.vector.tensor_scalar_mul(s_pp[mh][:], s_pp[mh][:], 127.0)
                nc.vector.tensor_scalar_mul(r1s[mh][:], rmax[mh][:],
                                            1.0 / 127.0)

            def pass1(mh, c, xc):
                # t1 = x*s + 2^23: fp32 RNE puts q = round(x*s) in the low
                # mantissa bits; the int16 view of t1 IS q (two's complement).
                t1 = t1p.tile([128, 2048], FP32, tag="t1", name=f"t1_{mh}{c}")
                nc.scalar.activation(
                    t1[:], xc[:], mybir.ActivationFunctionType.Copy,
                    bias=MAGIC, scale=s_pp[mh][:],
                )
                t1s[(mh, c)] = t1

            def transpose_pair(mh, pr):
                """Transpose k-blocks 8pr..8pr+7 of m-tile mh into one PSUM
                bank via int16 is_transpose matmuls; return psT for evac."""
                psT = pstp.tile([128, 1024], F16, tag="psT")
                for j in range(8):
                    b = 8 * pr + j
                    c, jj = b // 16, b % 16
                    # fp16 bitcast view: the transpose is a raw bit move, the
                    # evac below reinterprets the bits as int16
                    qv = t1s[(mh, c)].bitcast(F16).rearrange(
                        "p (kk two) -> p kk two", two=2)[:, :, 0]
                    nc.tensor.transpose(
                        psT[:, 128 * j : 128 * (j + 1)],
                        qv[:, 128 * jj : 128 * (jj + 1)], ident[:],
                    )
                return psT

            def evac_pair(mh, pr, psT, eng):
                dst = qtqp[(mh, pr)][:]
                src_i16 = psT[:].bitcast(I16)
                if eng == 0:
                    nc.scalar.activation(
                        dst, src_i16, mybir.ActivationFunctionType.Copy
                    )
                else:
                    nc.vector.tensor_copy(dst, src_i16)

            started = [[False] * nh_n for _ in range(mt)]

            def mains(mh, b, stop=False):
                u = b // 4
                wt3 = wtiles[u]
                lhsT = qtqp[(mh, b // 8)].rearrange(
                    "p (f mm) -> p f mm", f=8)[:, b % 8, :]
                for nh in range(nh_n):
                    nc.tensor.matmul(
                        psm[mh][nh][:],
                        lhsT,
                        wt3[:, b % 4, nsl * nh : nsl * (nh + 1)],
                        start=not started[mh][nh], stop=stop,
                    )
                    started[mh][nh] = True

            def finalize(mh):
                for nh in range(nh_n):
                    ob = obp.tile([128, nsl], BF16, tag="ob")
                    nc.scalar.activation(
                        ob[:], psm[mh][nh][:],
                        mybir.ActivationFunctionType.Copy, scale=r1s[mh][:],
                    )
                    nc.scalar.dma_start(
                        out_r[mh, :, nsl * nh : nsl * (nh + 1)], ob[:]
                    )

            # ---------------- emission schedule ----------------
            xs0 = load_x(0)
            # PE warms tied to x-mt0 chunk arrivals (fp32, dense)
            for c in range(ck):
                warm(xs0[c], 512, n_mm=2 if c < ck - 1 else 4, name=f"a{c}")
            rowmax(0, xs0)
            # W residents: gated so x streams at full bandwidth first
            gc = min(1, ck - 1)
            for u in range(min(2, n_res)):
                load_w(u, wresp, gate=rpart[0][:, gc : gc + 1])
            xs1 = load_x(1)
            for u in range(2, n_res):
                load_w(u, wresp, gate=rmax[0])
            # mt0 quant + transposes; pairs 0-1 evac on DVE early (feed P1)
            psTs = {}
            for c in range(ck):
                pass1(0, c, xs0[c])
                if c == 0:
                    warm(t1s[(0, 0)], 512, n_mm=3, name="t1")
            for pr in range(2):
                psT = transpose_pair(0, pr)
                evac_pair(0, pr, psT, eng=1)
            rowmax(1, xs1)
            for u in range(n_res, min(n_res + 4, wu)):
                load_w(u, wstrp, gate=rmax[1])
            # P1: early mh0 mains on resident W (b0..15), interleaved with
            # the rest of the mt0 transposes
            for pr in range(2, npr):
                psT = transpose_pair(0, pr)
                evac_pair(0, pr, psT, eng=1)
                for b in range(8 * (pr - 2), min(8 * (pr - 1), bres)):
                    mains(0, b, stop=(wu == n_res and b == kb - 1))
            for b in range(8 * (npr - 2), bres):
                mains(0, b, stop=(wu == n_res and b == kb - 1))
            # mt1 quant + transposes (evacs alternate DVE/ACT)
            for c in range(ck):
                pass1(1, c, xs1[c])
            for pr in range(npr):
                psT = transpose_pair(1, pr)
                evac_pair(1, pr, psT, eng=pr % 2)
            for u in range(n_res + 4, wu):
                load_w(u, wstrp)
            # P3: mh1 catch-up on resident W (before P2 so both psm tile sets
            # finish accumulation at the end of P2)
            for b in range(0, bres):
                mains(1, b, stop=(wu == n_res and b == kb - 1))
            # P2: paired mains on streamed W
            for b in range(bres, kb):
                mains(0, b, stop=(b == kb - 1))
                mains(1, b, stop=(b == kb - 1))
            finalize(0)
            finalize(1)
    nc.compile()
    return nc


def host_prep(input, weight_scale, weight, ns):
    """Shard + relayout inputs for each core: decode the packed 2-bit ternary
    codes and fold the per-(row, group) scale into a full bf16 W[k, n] per
    core (pure static-weight relayout), plus fp32 activation passthrough."""
    n, k4 = weight.shape
    k = k4 * 4
    x = np.ascontiguousarray(input, dtype=np.float32)
    w_bytes = weight.astype(np.uint8)                       # [N, K/4]
    codes = np.empty((n, k), dtype=np.int8)                 # [N, K] in {-1,0,1}
    for j in range(4):
        codes[:, j::4] = ((w_bytes >> (2 * j)) & 3).astype(np.int8) - 1
    ws2 = np.asarray(weight_scale, dtype=np.float32).reshape(n, -1)  # [N, K/GS]
    ws2_b = ws2.astype(ml_dtypes.bfloat16)
    # W[n, k] = codes * ws (exact in bf16: +-1 * bf16 scale)
    wf = codes.astype(np.float32) * ws2_b.astype(np.float32).repeat(GS, axis=1)
    wf = wf.astype(ml_dtypes.bfloat16)
    in_maps = []
    for c in range(n // ns):
        sl = slice(c * ns, (c + 1) * ns)
        wf_c = np.ascontiguousarray(wf[sl].T)               # [K, ns] bf16
        in_maps.append({"x": x, "wf": wf_c})
    return in_maps


_NC_CACHE = {}


def _get_nc(m, k, ns):
    key = (m, k, ns)
    if key not in _NC_CACHE:
        _NC_CACHE[key] = build_nc(m, k, ns)
    return _NC_CACHE[key]


def kernel(input, weight_scale, weight, group_size=GS, trace=False):
    m, k = input.shape
    n = weight.shape[0]
    ns = n // NCORES
    nc = _get_nc(m, k, ns)
    in_maps = host_prep(input, weight_scale, weight, ns)
    res = bass_utils.run_bass_kernel_spmd(
        nc, in_maps, core_ids=list(range(NCORES)), trace=trace
    )
    out = np.concatenate([r["out"] for r in res.results], axis=1)
    if trace:
        return out, res
    return out


if __name__ == "__main__":
    # small-config CoreSim check
    from concourse.bass_interp import CoreSim

    rng = np.random.default_rng(0)
    m, k, ns = 256, 2048, 256
    x = rng.standard_normal((m, k), dtype=np.float32)
    w_tern = rng.integers(-1, 2, size=(ns, k)).astype(np.int32)
    codes = (w_tern + 1).reshape(ns, k // 4, 4)
    packed = (
        codes[..., 0] | (codes[..., 1] << 2) | (codes[..., 2] << 4)
        | (codes[..., 3] << 6)
    ).astype(np.int32)
    ws = rng.uniform(0.001, 0.02, size=(ns, k // GS, 1)).astype(np.float32)

    # numpy reference
    s = 127.0 / np.clip(np.abs(x).max(axis=-1, keepdims=True), 1e-5, None)
    q = np.clip(np.round(x * s), -128, 127)
    wf = w_tern.astype(np.float32) * np.repeat(ws.reshape(ns, -1), GS, axis=1)
    ref = ((q @ wf.T) / s).astype(ml_dtypes.bfloat16).astype(np.float32)

    nc = build_nc(m, k, ns)
    im = host_prep(x, ws, packed, ns)[0]
    sim = CoreSim(nc)
    for kk, v in im.items():
        sim.tensor(kk)[:] = v
    sim.simulate()
    got = np.asarray(sim.tensor("out")).astype(np.float32)
    err = np.abs(got - ref).max() / (np.abs(ref).max() + 1e-9)
    print("rel err (absmax):", err)
    rms = np.sqrt(((got - ref) ** 2).mean()) / (np.sqrt((ref**2).mean()) + 1e-9)
    print("rel err (rms):", rms)
